# revision 1
# baseline (speedup 1.0000x reference)
"""Trainium2 Bass kernel for nn_CNN3_FPB (dense CNN + bypass MLP + FC head).

Data-parallel over 8 NeuronCores: batch 2048 -> 256 per core. All weights
replicated. Inside each core:

  stage0: y[p,b,c] = Wp1 @ xT[:,b,c] + Wp0 @ x0[:,b] + bp, relu
          (single K=128 matmul: [Wp1T; Wp0T] stacked against [xT; x0bcast])
  conv1:  K=3 stride 1 -> 3 accumulated matmuls over psum, K(contract)=64
  conv2:  K=3 stride 2, Cin=128, Cout=256 (2 M-chunks)
  conv3:  K=3 stride 2, Cin=256 (2 K-chunks), Cout=256 (2 M-chunks)
  fc1:    position-wise contraction (64 pos x 2 ci-chunks x 8 f-chunks),
          bf16 weights streamed from HBM, bf16 activations (zres)
  fc2:    contraction over f (8 chunks), fp32r

Activation layout: [channels(part), batch, position] with one zero pad col
on each side of the position axis so conv taps are plain shifted AP reads.
All fp32 matmuls are issued as float32r (full-rate at N>=256).
"""

import os
import sys
from contextlib import ExitStack

import numpy as np

for _p in ("/opt/trn_rl_repo", "/root/.axon_site/_ro/trn_rl_repo"):
    if os.path.isdir(_p) and _p not in sys.path:
        sys.path.insert(0, _p)

import ml_dtypes  # noqa: E402
import concourse.bass as bass  # noqa: E402
from concourse import bacc  # noqa: E402
import concourse.mybir as mybir  # noqa: E402
import concourse.tile as tile  # noqa: E402

F32 = mybir.dt.float32
F32R = mybir.dt.float32r
BF16 = mybir.dt.bfloat16
RELU = mybir.ActivationFunctionType.Relu
ADD = mybir.AluOpType.add
MAX = mybir.AluOpType.max

# Problem constants (hardcoded; must match the grading problem).
B, CL, IL = 2048, 256, 64
NCORES = 8
BC = B // NCORES  # 256 samples per core
BB = 16           # samples per conv block
NBLK = BC // BB
PC = 64
CH1, CH2, CH3 = 128, 256, 256
L1, L2, L3 = 255, 128, 64
F1 = 1024
OUTC = 2

NBIAS = 19  # bias columns: see _prep_bias

# Trunk (stage0..conv3) matmul dtype: F32R (tf32-ish) or BF16.
TRUNK_BF16 = True


def _r(ap):
    return ap.bitcast(F32R)


def build_nc():
    nc = bacc.Bacc()
    TDT = BF16 if TRUNK_BF16 else F32R

    def memset_pad(ap):
        if TRUNK_BF16:
            nc.gpsimd.memset(ap, 0.0)
        else:
            nc.gpsimd.memset(ap.bitcast(F32), 0.0)

    xs = nc.declare_dram_parameter("xs", [NBLK, 128, CL, BB], TDT, isOutput=False)
    x0s = nc.declare_dram_parameter("x0s", [64, BC], F32R, isOutput=False)
    wstk = nc.declare_dram_parameter("wstk", [128, 64], TDT, isOutput=False)
    w1 = nc.declare_dram_parameter("w1", [64, 3, CH1], TDT, isOutput=False)
    w2 = nc.declare_dram_parameter("w2", [128, 3, CH2], TDT, isOutput=False)
    w3 = nc.declare_dram_parameter("w3", [128, 2, 3, CH3], TDT, isOutput=False)
    wb1 = nc.declare_dram_parameter("wb1", [64, 64], F32R, isOutput=False)
    wb2 = nc.declare_dram_parameter("wb2", [64, 128], F32R, isOutput=False)
    wb3 = nc.declare_dram_parameter("wb3", [128, 256], F32R, isOutput=False)
    wg = nc.declare_dram_parameter("wg", [L3, 128, 2, F1], BF16, isOutput=False)
    wbyp = nc.declare_dram_parameter("wbyp", [128, 2, F1], BF16, isOutput=False)
    wfc2 = nc.declare_dram_parameter("wfc2", [128, 8, OUTC], F32R, isOutput=False)
    bias = nc.declare_dram_parameter("bias", [128, NBIAS], F32, isOutput=False)
    out = nc.declare_dram_parameter("out", [OUTC, BC], F32, isOutput=True)

    with ExitStack() as ctx:
        tc = ctx.enter_context(tile.TileContext(nc))
        wpool = ctx.enter_context(tc.tile_pool(name="wpool", bufs=1))
        xpool = ctx.enter_context(tc.tile_pool(name="xpool", bufs=2))
        h1pool = ctx.enter_context(tc.tile_pool(name="h1pool", bufs=2))
        h2pool = ctx.enter_context(tc.tile_pool(name="h2pool", bufs=2))
        h3pool = ctx.enter_context(tc.tile_pool(name="h3pool", bufs=2))
        zpool = ctx.enter_context(tc.tile_pool(name="zpool", bufs=1))
        wgpool = ctx.enter_context(tc.tile_pool(name="wgpool", bufs=8))
        spool = ctx.enter_context(tc.tile_pool(name="spool", bufs=1))

        # ---- persistent weights / bias (startup-critical DMAs first) ----
        x0_t = wpool.tile([64, BC], F32R)
        nc.sync.dma_start(x0_t[:], x0s[:])
        bias_t = wpool.tile([128, NBIAS], F32)
        nc.sync.dma_start(bias_t[:], bias[:])
        wstk_t = wpool.tile([128, 64], TDT)
        nc.sync.dma_start(wstk_t[:], wstk[:])
        wb1_t = wpool.tile([64, 64], F32R)
        nc.sync.dma_start(wb1_t[:], wb1[:])
        wb2_t = wpool.tile([64, 128], F32R)
        nc.sync.dma_start(wb2_t[:], wb2[:])
        wb3_t = wpool.tile([128, 256], F32R)
        nc.sync.dma_start(wb3_t[:], wb3[:])
        # prefetch x for the first two blocks before the heavy weight DMAs
        xt_pre = {}
        for blk in range(2):
            t = xpool.tile([128, CL, BB], TDT, name=f"xt{blk}", tag="xt")
            nc.sync.dma_start(t[:], xs[blk, :, :, :])
            xt_pre[blk] = t
        w1_t = wpool.tile([64, 3, CH1], TDT)
        nc.sync.dma_start(w1_t[:], w1[:])
        w2_t = wpool.tile([128, 3, CH2], TDT)
        nc.sync.dma_start(w2_t[:], w2[:])
        w3_t = wpool.tile([128, 2, 3, CH3], TDT)
        nc.sync.dma_start(w3_t[:], w3[:])
        wfc2_t = wpool.tile([128, 8, OUTC], F32R)
        nc.sync.dma_start(wfc2_t[:], wfc2[:])

        bp_ap = bias_t[:64, 0:1]
        b1_ap = bias_t[:, 1:2]

        # ---- bypass MLP (tiny) ----
        cpsum_ctx = ExitStack()
        cpsum = cpsum_ctx.enter_context(tc.tile_pool(name="cpsum", bufs=4, space="PSUM"))
        ps = cpsum.tile([64, BC], F32, tag="ps")
        nc.tensor.matmul(ps[:], (wb1_t[:]), (x0_t[:]), start=True, stop=True)
        s1 = spool.tile([64, BC], F32R)
        nc.scalar.activation(s1[:], ps[:], RELU, bias=bias_t[:64, 6:7])
        ps = cpsum.tile([128, BC], F32, tag="ps")
        nc.tensor.matmul(ps[:], (wb2_t[:]), (s1[:]), start=True, stop=True)
        s2 = spool.tile([128, BC], F32R)
        nc.scalar.activation(s2[:], ps[:], RELU, bias=bias_t[:, 7:8])
        fbyp = spool.tile([128, 2, BC], BF16)
        for m in range(2):
            ps = cpsum.tile([128, BC], F32, tag="ps")
            nc.tensor.matmul(
                ps[:], (wb3_t[:, m * 128 : (m + 1) * 128]), (s2[:]),
                start=True, stop=True,
            )
            nc.vector.tensor_scalar(
                fbyp[:, m, :], ps[:], bias_t[:, 8 + m : 9 + m], 0.0, ADD, MAX
            )

        # ---- resident conv3 output (fc1 rhs), bf16: [ci, cich, l3, b] ----
        zres = zpool.tile([128, 2, L3, BC], BF16)

        # ---- conv trunk, per batch block ----
        # Layout: [channels(part), position, batch] (batch innermost) so every
        # conv tap reads a contiguous 2D span (fp32r matmul requirement).
        # Stride-2 conv inputs are parity-split (even/odd position tensors).
        # Fine-grained chunks (one PSUM bank each) keep the PE pipeline deep.
        S0_CHUNKS = [(1 + 32 * j, 32 if j < 7 else 31) for j in range(8)]
        C1_CHUNKS = [(32 * j, 32 if j < 7 else 31) for j in range(8)]
        for blk in range(NBLK):
            b0 = blk * BB

            if blk in xt_pre:
                xt = xt_pre[blk]
            else:
                xt = xpool.tile([128, CL, BB], TDT, name="xt", tag="xt")
                nc.sync.dma_start(xt[:], xs[blk, :, :, :])

            # stage0 -> h1 [64, 257, BB]: col j = pos j-1 (c=j); pads j=0,256
            h1 = h1pool.tile([64, L1 + 2, BB], TDT)
            memset_pad(h1[:, 0:1, :])
            memset_pad(h1[:, 256:257, :])
            for pair in range(4):
                ps = cpsum.tile([64, 2 * 32 * BB], F32, tag="ps")
                tot = 0
                for i, (c0, cc) in enumerate(S0_CHUNKS[2 * pair : 2 * pair + 2]):
                    nc.tensor.matmul(
                        ps[:, 512 * i : 512 * i + cc * BB], wstk_t[:],
                        xt[:, c0 : c0 + cc, :].rearrange("p c b -> p (c b)"),
                        start=True, stop=True,
                    )
                    tot += cc
                cp0 = S0_CHUNKS[2 * pair][0]
                nc.scalar.activation(
                    h1[:, cp0 : cp0 + tot, :].rearrange("p c b -> p (c b)"),
                    ps[:, : tot * BB], RELU, bias=bp_ap,
                )

            # conv1 -> h2 parity-split: h2e [128,128,BB] (pos 0,2,..254),
            # h2o [128,129,BB] (j=(pos+1)/2 for odd pos -1..255; pads j=0,128)
            h2e = h2pool.tile([128, 128, BB], TDT)
            h2o = h2pool.tile([128, 129, BB], TDT)
            memset_pad(h2o[:, 0:1, :])
            memset_pad(h2o[:, 128:129, :])
            for pair in range(4):
                ps = cpsum.tile([128, 2 * 32 * BB], F32, tag="ps")
                ltot = 0
                for i, (l0, lc) in enumerate(C1_CHUNKS[2 * pair : 2 * pair + 2]):
                    for k in range(3):
                        nc.tensor.matmul(
                            ps[:, 512 * i : 512 * i + lc * BB], w1_t[:, k, :],
                            h1[:, l0 + k : l0 + k + lc, :]
                            .rearrange("p l b -> p (l b)"),
                            start=(k == 0), stop=(k == 2),
                        )
                    ltot += lc
                lp0 = C1_CHUNKS[2 * pair][0]
                ps3 = ps.rearrange("p (t x) -> p t x", x=32)
                ne, no = (ltot + 1) // 2, ltot // 2
                nc.vector.tensor_scalar(
                    h2e[:, lp0 // 2 : lp0 // 2 + ne, :], ps3[:, :ne, 0:16],
                    b1_ap, 0.0, ADD, MAX,
                )
                nc.scalar.activation(
                    h2o[:, lp0 // 2 + 1 : lp0 // 2 + 1 + no, :], ps3[:, :no, 16:32],
                    RELU, bias=b1_ap,
                )

            # conv2 -> h3 parity-split per ci-chunk: h3e [128,2,64,BB],
            # h3o [128,2,65,BB] (j=(pos+1)/2 for odd pos -1..127; pad j=0)
            h3e = h3pool.tile([128, 2, 64, BB], TDT)
            h3o = h3pool.tile([128, 2, 65, BB], TDT)
            memset_pad(h3o[:, :, 0:1, :])
            for m in range(2):
                for pair in range(2):
                    ps = cpsum.tile([128, 2 * 32 * BB], F32, tag="ps")
                    for i in range(2):
                        l20 = 64 * pair + 32 * i
                        for k in range(3):
                            if k == 0:
                                rhs = h2o[:, l20 : l20 + 32, :]
                            elif k == 1:
                                rhs = h2e[:, l20 : l20 + 32, :]
                            else:
                                rhs = h2o[:, l20 + 1 : l20 + 33, :]
                            nc.tensor.matmul(
                                ps[:, 512 * i : 512 * (i + 1)],
                                w2_t[:, k, m * 128 : (m + 1) * 128],
                                rhs.rearrange("p l b -> p (l b)"),
                                start=(k == 0), stop=(k == 2),
                            )
                    ps3 = ps.rearrange("p (t x) -> p t x", x=32)
                    nc.scalar.activation(
                        h3e[:, m, 32 * pair : 32 * pair + 32, :], ps3[:, :, 0:16],
                        RELU, bias=bias_t[:, 2 + m : 3 + m],
                    )
                    nc.vector.tensor_scalar(
                        h3o[:, m, 32 * pair + 1 : 32 * pair + 33, :],
                        ps3[:, :, 16:32], bias_t[:, 2 + m : 3 + m], 0.0, ADD, MAX,
                    )

            # conv3 -> zres[:, m, l3, b] (bf16)
            for m in range(2):
                ps = cpsum.tile([128, 2 * 32 * BB], F32, tag="ps")
                for q in range(2):
                    l30 = 32 * q
                    acc = 0
                    for c in range(2):
                        for k in range(3):
                            if k == 0:
                                rhs = h3o[:, c, l30 : l30 + 32, :]
                            elif k == 1:
                                rhs = h3e[:, c, l30 : l30 + 32, :]
                            else:
                                rhs = h3o[:, c, l30 + 1 : l30 + 33, :]
                            nc.tensor.matmul(
                                ps[:, 512 * q : 512 * (q + 1)],
                                w3_t[:, c, k, m * 128 : (m + 1) * 128],
                                rhs.rearrange("p l b -> p (l b)"),
                                start=(acc == 0), stop=(acc == 5),
                            )
                            acc += 1
                ps3 = ps.rearrange("p (l b) -> p l b", b=BB)
                if m == 0:
                    nc.scalar.activation(
                        zres[:, m, :, b0 : b0 + BB], ps3[:],
                        RELU, bias=bias_t[:, 4 + m : 5 + m],
                    )
                else:
                    nc.vector.tensor_scalar(
                        zres[:, m, :, b0 : b0 + BB], ps3[:],
                        bias_t[:, 4 + m : 5 + m], 0.0, ADD, MAX,
                    )

        # ---- fc1: stream bf16 weight slabs, accumulate 130 matmuls/f-chunk ----
        cpsum_ctx.close()
        fpsum_ctx = ExitStack()
        fpsum = fpsum_ctx.enter_context(tc.tile_pool(name="fpsum", bufs=1, space="PSUM"))
        fps = [
            fpsum.tile([128, BC], F32, tag=f"fps{i}", name=f"fps{i}")
            for i in range(8)
        ]
        for l3 in range(L3):
            slab = wgpool.tile([128, 2, F1], BF16, tag="slab")
            nc.sync.dma_start(slab[:], wg[l3, :, :, :])
            for c in range(2):
                first = l3 == 0 and c == 0
                for f in range(8):
                    nc.tensor.matmul(
                        fps[f][:],
                        slab[:, c, f * 128 : (f + 1) * 128],
                        zres[:, c, l3, :],
                        start=first, stop=False,
                    )
        slabb = wgpool.tile([128, 2, F1], BF16, tag="slab")
        nc.sync.dma_start(slabb[:], wbyp[:, :, :])
        for c in range(2):
            for f in range(8):
                nc.tensor.matmul(
                    fps[f][:],
                    slabb[:, c, f * 128 : (f + 1) * 128],
                    fbyp[:, c, :],
                    start=False, stop=(c == 1),
                )

        z2 = spool.tile([128, 8, BC], F32R)
        for f in range(8):
            nc.scalar.activation(
                z2[:, f, :], fps[f][:], RELU,
                bias=bias_t[:, 10 + f : 11 + f],
            )

        # ---- fc2 ----
        fpsum_ctx.close()
        f2psum_ctx = ExitStack()
        f2psum = f2psum_ctx.enter_context(tc.tile_pool(name="f2psum", bufs=1, space="PSUM"))
        ps = f2psum.tile([2, BC], F32, tag="ps2", name="ps2")
        for f in range(8):
            nc.tensor.matmul(
                ps[:], (wfc2_t[:, f, :]), (z2[:, f, :]),
                start=(f == 0), stop=(f == 7),
            )
        osb = spool.tile([2, BC], F32)
        nc.vector.tensor_scalar_add(osb[:], ps[:], bias_t[:2, 18:19])
        nc.sync.dma_start(out[:], osb[:])
        f2psum_ctx.close()

    nc.compile()
    return nc


def _prep_inputs(inputs):
    """Host-side layout prep. Returns (shared weight map, per-core input maps)."""
    f32 = lambda a: np.ascontiguousarray(np.asarray(a), dtype=np.float32)
    x = f32(inputs["x"])
    Wp = f32(inputs["Wp"])
    W1, W2, W3 = f32(inputs["W1"]), f32(inputs["W2"]), f32(inputs["W3"])
    Wb1, Wb2, Wb3 = f32(inputs["Wb1"]), f32(inputs["Wb2"]), f32(inputs["Wb3"])
    Wfc1, Wfc2 = f32(inputs["Wfc1"]), f32(inputs["Wfc2"])

    xr3 = x.reshape(B, CL, IL)  # [b, c, i]
    xT = np.ascontiguousarray(xr3.transpose(2, 1, 0))  # [i, c, b]
    x0T = np.ascontiguousarray(xr3[:, 0, :].T)  # [i, b]

    tnp = ml_dtypes.bfloat16 if TRUNK_BF16 else np.float32
    shared = {
        "wstk": np.ascontiguousarray(
            np.concatenate([Wp[:, :, 1].T, Wp[:, :, 0].T], axis=0)
        ).astype(tnp),
        "w1": np.ascontiguousarray(W1.transpose(1, 2, 0)).astype(tnp),
        "w2": np.ascontiguousarray(W2.transpose(1, 2, 0)).astype(tnp),
        "w3": np.ascontiguousarray(
            W3.transpose(1, 2, 0).reshape(2, 128, 3, CH3).transpose(1, 0, 2, 3)
        ).astype(tnp),
        "wb1": np.ascontiguousarray(Wb1.T),
        "wb2": np.ascontiguousarray(Wb2.T),
        "wb3": np.ascontiguousarray(Wb3.T),
        "wg": np.ascontiguousarray(
            Wfc1[:, : CH3 * L3].reshape(F1, CH3, L3).transpose(2, 1, 0)
            .reshape(L3, 2, 128, F1).transpose(0, 2, 1, 3)
        ).astype(ml_dtypes.bfloat16),
        "wbyp": np.ascontiguousarray(
            Wfc1[:, CH3 * L3 :].T.reshape(2, 128, F1).transpose(1, 0, 2)
        ).astype(ml_dtypes.bfloat16),
        "wfc2": np.ascontiguousarray(
            Wfc2.T.reshape(8, 128, OUTC).transpose(1, 0, 2)
        ),
    }

    bias_np = np.zeros((128, NBIAS), np.float32)
    bias_np[:64, 0] = f32(inputs["bp"])
    bias_np[64:, 0] = f32(inputs["bp"])
    bias_np[:, 1] = f32(inputs["b1"])
    b2, b3 = f32(inputs["b2"]), f32(inputs["b3"])
    bias_np[:, 2], bias_np[:, 3] = b2[:128], b2[128:]
    bias_np[:, 4], bias_np[:, 5] = b3[:128], b3[128:]
    bias_np[:64, 6] = f32(inputs["bb1"])
    bias_np[:, 7] = f32(inputs["bb2"])
    bb3 = f32(inputs["bb3"])
    bias_np[:, 8], bias_np[:, 9] = bb3[:128], bb3[128:]
    bias_np[:, 10:18] = f32(inputs["bfc1"]).reshape(8, 128).T
    bias_np[:2, 18] = f32(inputs["bfc2"])
    shared["bias"] = bias_np

    in_maps = []
    for core in range(NCORES):
        sl = slice(core * BC, (core + 1) * BC)
        xc = xT[:, :, sl].reshape(IL, CL, NBLK, BB)
        x0b = x0T[:, sl].reshape(IL, NBLK, BB)
        xs_core = np.empty((NBLK, 128, CL, BB), tnp)
        xs_core[:, :64] = xc.transpose(2, 0, 1, 3)
        xs_core[:, 64:] = x0b.transpose(1, 0, 2)[:, :, None, :]
        m = dict(shared)
        m["xs"] = xs_core
        m["x0s"] = np.ascontiguousarray(x0T[:, sl])
        in_maps.append(m)
    return in_maps


_NC_CACHE = {}


def _get_nc():
    if "nc" not in _NC_CACHE:
        _NC_CACHE["nc"] = build_nc()
    return _NC_CACHE["nc"]


def run(inputs, trace=False):
    from concourse.bass_utils import run_bass_kernel_spmd

    nc = _get_nc()
    in_maps = _prep_inputs(inputs)
    res = run_bass_kernel_spmd(
        nc, in_maps, core_ids=list(range(NCORES)), trace=trace
    )
    outs = [np.asarray(r["out"]) for r in res.results]
    full = np.concatenate([o.T for o in outs], axis=0).astype(np.float32)
    return full, res


def kernel(**inputs) -> np.ndarray:
    full, _ = run(inputs, trace=False)
    return full

